# revision 1
# baseline (speedup 1.0000x reference)
"""ConvergedInhibition TRN2 kernel.

The reference computes, per pixel (n,h,w), an FFT deconvolution along the
channel axis: y = ifft(fft(x)/fft(k)).real. Since k is fixed, this is a
circular convolution with g = ifft(1/fft(k)): y[i] = sum_j g[(i-j) mod C] x[j]
— a dense CxC circulant matmul applied to every pixel. Viewing activations[n]
as a [C, H*W] matrix A_n, the problem is out_n = G @ A_n: a [512,512] x
[512,3136] matmul per image, data-parallel over 32 images across 8 cores.

Implementation choices (measured on HW):
- fp16 I/O: activations/weights are rounded to fp16 on the host and the
  output is stored as fp16 (upcast on host). This halves HBM traffic, which
  is the roofline here, and costs ~2^-11 relative rounding (~3.6e-4 total).
- The deconv kernel g is concentrated in a ~224-wide circular window around
  t=288 (the reference center-pads k, shifting the delta to position 224).
  Rotating output rows by S=288 (z[r] = y[(r+S) mod C]) aligns the support
  so that only 3 of 4 K-chunks of the contraction carry mass; the 4th is
  dropped (adds ~7e-5 error). The rotation is undone by a host-side gather.
- Matmuls run at full PE rate in fp16, contracting K=3x128 into fp32 PSUM.
"""

import numpy as np

import concourse.bass as bass  # noqa: F401  (registers bass types)
import concourse.mybir as mybir
import concourse.tile as tile
from concourse import bacc
from concourse.bass_utils import run_bass_kernel_spmd

N_CORES = 8
N, C, H, W = 32, 512, 56, 56
HW = H * W                      # 3136
IMGS = N // N_CORES             # 4 images per core
P = 128                         # partitions
NCHUNK = C // P                 # 4
PT = 392                        # pixel tile (free dim), 3136 = 8*392
NPT = HW // PT                  # 8
CB = 784                        # DMA column block, 3136 = 4*784
NCB = HW // CB                  # 4
ROT = 288                       # output-row rotation aligning g's support
KEPT_D = (0, 1, 2)              # kept (zc - jc) mod 4 chunk distances
IO_DT = mybir.dt.float16
IO_NP = np.float16

_CACHE = {}

RAW = True  # hand-rolled bacc kernel (V4); False = TileContext fallback (V3)


def _build_nc_raw():
    """Hand-rolled engine programs with explicit semaphores.

    Same dataflow as the Tile version, but without Tile's ~6us semaphore-init
    preamble and ~9us reset/barrier epilogue. Streams:
      Sync:   act loads (img, cb, jc) + half the stores, one HWDGE ring
      Scalar: gt loads + the other half of the stores, the other ring
      Tensor: 3-matmul PSUM groups per (img, cb, zc, p2) tile
      Vector: PSUM->fp16 casts into o_sb
    o_sb is per-(img, zc) (no reuse) so stores never gate casts; a_sb is
    double-buffered over images gated by s_mm; the 8 PSUM banks are a ring
    gated by s_cast.
    """
    nc = bacc.Bacc("TRN2", target_bir_lowering=False, debug=False,
                   num_devices=N_CORES)
    act = nc.dram_tensor("act", [IMGS, C, HW], IO_DT, kind="ExternalInput")
    gt = nc.dram_tensor("gt", [C, C], IO_DT, kind="ExternalInput")
    out = nc.dram_tensor("out", [IMGS, C, HW], IO_DT, kind="ExternalOutput")

    act_v = act.ap().rearrange("n (jc p) m -> n jc p m", p=P)
    gt_v = gt.ap().rearrange("(jc p) r -> jc p r", p=P)
    out_v = out.ap().rearrange("n (zc p) m -> n zc p m", p=P)

    NKEPT = len(KEPT_D)
    P2 = NPT // NCB                       # p-tiles per column block (2)
    TILES_PER_CB = NCHUNK * P2            # 8 psum tiles per (img, cb)
    TILES_PER_IMG = NCB * TILES_PER_CB    # 32

    def tidx(img, cb, zc, p2):
        return img * TILES_PER_IMG + cb * TILES_PER_CB + zc * P2 + p2

    def store_ring(cb, zc):
        return "sync" if (cb + zc) % 2 == 0 else "scalar"

    from contextlib import ExitStack
    with ExitStack() as ctx:
        a_sb = [ctx.enter_context(
            nc.sbuf_tensor(f"a_sb{h}", [P, NCHUNK * HW], IO_DT)).ap()
            for h in range(2)]
        gt_sb = ctx.enter_context(
            nc.sbuf_tensor("gt_sb", [P, NCHUNK * C], IO_DT)).ap()
        o_sb = [[ctx.enter_context(
            nc.sbuf_tensor(f"o_sb{i}_{z}", [P, HW], IO_DT)).ap()
            for z in range(NCHUNK)] for i in range(IMGS)]
        psum = [ctx.enter_context(
            nc.psum_tensor(f"ps{i}", [P, 512], mybir.dt.float32)).ap()
            for i in range(8)]

        s_gt = nc.alloc_semaphore("s_gt")
        s_ld = [[nc.alloc_semaphore(f"s_ld{h}_{cb}") for cb in range(NCB)]
                for h in range(2)]
        # gpsimd (SWDGE) loads need their own sems: a sem driven by a
        # software DMA can't also be updated by HWDGE
        s_ldg = [nc.alloc_semaphore(f"s_ldg_{cb}") for cb in range(NCB)]
        s_mm = nc.alloc_semaphore("s_mm")
        s_cast = nc.alloc_semaphore("s_cast")
        s_st = {"sync": nc.alloc_semaphore("s_st_sync"),
                "scalar": nc.alloc_semaphore("s_st_sca")}
        all_sems = ([s_gt, s_mm, s_cast, s_st["sync"], s_st["scalar"]]
                    + [s for row in s_ld for s in row] + s_ldg)

        # Stage 0: clear semaphores; the Block-exit barrier orders this
        # before any use in the main block (sems are NOT zeroed on alloc
        # and must not carry values across executions).
        with nc.Block("clears") as blk:

            @blk.sync
            def _(sync):
                for s in all_sems:
                    sync.sem_clear(s)

        with nc.Block("main") as blk:

            def emit_loads(sync, img, cb):
                if img >= 2:
                    sync.wait_ge(s_mm, TILES_PER_IMG * (img - 2)
                                 + TILES_PER_CB * (cb + 1))
                for jc in range(NCHUNK):
                    sync.dma_start(
                        a_sb[img % 2][
                            :, jc * HW + cb * CB: jc * HW + (cb + 1) * CB],
                        act_v[img, jc, :, cb * CB:(cb + 1) * CB],
                    ).then_inc(s_ld[img % 2][cb], 16)

            @blk.sync
            def _(sync):
                n_store = 0
                for img in range(min(2, IMGS)):
                    for cb in range(NCB):
                        emit_loads(sync, img, cb)
                for img in range(IMGS):
                    for cb in range(NCB):
                        for zc in range(NCHUNK):
                            if store_ring(cb, zc) != "sync":
                                continue
                            sync.wait_ge(s_cast,
                                         tidx(img, cb, zc, P2 - 1) + 1)
                            sync.dma_start(
                                out_v[img, zc, :, cb * CB:(cb + 1) * CB],
                                o_sb[img][zc][:, cb * CB:(cb + 1) * CB],
                            ).then_inc(s_st["sync"], 16)
                            n_store += 1
                        if img + 2 < IMGS:
                            emit_loads(sync, img + 2, cb)
                sync.wait_ge(s_st["sync"], 16 * n_store)

            @blk.scalar
            def _(scalar):
                for jc in range(NCHUNK):
                    scalar.dma_start(
                        gt_sb[:, jc * C:(jc + 1) * C], gt_v[jc],
                    ).then_inc(s_gt, 16)
                n_store = 0
                for img in range(IMGS):
                    for cb in range(NCB):
                        for zc in range(NCHUNK):
                            if store_ring(cb, zc) != "scalar":
                                continue
                            scalar.wait_ge(
                                s_cast, tidx(img, cb, zc, P2 - 1) + 1)
                            scalar.dma_start(
                                out_v[img, zc, :, cb * CB:(cb + 1) * CB],
                                o_sb[img][zc][:, cb * CB:(cb + 1) * CB],
                            ).then_inc(s_st["scalar"], 16)
                            n_store += 1
                scalar.wait_ge(s_st["scalar"], 16 * n_store)

            @blk.tensor
            def _(tensor):
                tensor.wait_ge(s_gt, 16 * NCHUNK)
                # HAM warmup while the first act loads land: ~12 matmuls on
                # gt data into bank 7 (overwritten by the first real group
                # before its first read; start=True resets accumulation)
                for _ in range(12):
                    tensor.matmul(psum[7][:, :PT], gt_sb[:, :P],
                                  gt_sb[:, :PT], start=True, stop=True)
                for img in range(IMGS):
                    for cb in range(NCB):
                        tensor.wait_ge(s_ld[img % 2][cb],
                                       64 * (img // 2 + 1))
                        for zc in range(NCHUNK):
                            for p2 in range(P2):
                                t = tidx(img, cb, zc, p2)
                                if t >= 8:
                                    tensor.wait_ge(s_cast, t - 7)
                                p = cb * P2 + p2
                                for i, d in enumerate(KEPT_D):
                                    jc = (zc - d) % NCHUNK
                                    mm = tensor.matmul(
                                        psum[t % 8][:, :PT],
                                        gt_sb[:, jc * C + zc * P:
                                              jc * C + (zc + 1) * P],
                                        a_sb[img % 2][
                                            :, jc * HW + p * PT:
                                            jc * HW + (p + 1) * PT],
                                        start=(i == 0), stop=(i == NKEPT - 1),
                                    )
                                mm.then_inc(s_mm)

            @blk.vector
            def _(vector):
                for img in range(IMGS):
                    for cb in range(NCB):
                        for zc in range(NCHUNK):
                            for p2 in range(P2):
                                t = tidx(img, cb, zc, p2)
                                vector.wait_ge(s_mm, t + 1)
                                p = cb * P2 + p2
                                vector.tensor_copy(
                                    o_sb[img][zc][:, p * PT:(p + 1) * PT],
                                    psum[t % 8][:, :PT],
                                ).then_inc(s_cast)

    nc.compile()
    return nc


def _build_nc():
    if RAW:
        return _build_nc_raw()
    return _build_nc_tile()


def _build_nc_tile():
    nc = bacc.Bacc("TRN2", target_bir_lowering=False, debug=False,
                   num_devices=N_CORES)
    act = nc.dram_tensor("act", [IMGS, C, HW], IO_DT, kind="ExternalInput")
    gt = nc.dram_tensor("gt", [C, C], IO_DT, kind="ExternalInput")
    out = nc.dram_tensor("out", [IMGS, C, HW], IO_DT, kind="ExternalOutput")

    with tile.TileContext(nc) as tc:
        with (
            tc.tile_pool(name="gtp", bufs=1) as gtp,
            tc.tile_pool(name="apool", bufs=3) as apool,
            tc.tile_pool(name="opool", bufs=2) as opool,
            tc.tile_pool(name="ps", bufs=8, space="PSUM") as psp,
        ):
            # gt_sb cols [jc*C + zc*P : ...] hold GTs[jc*P:(jc+1)*P, zc*P:...]:
            # the stationary operand for psum[zc] += blk.T @ x[jc].
            # gt loads go on the scalar ring so the first act loads aren't
            # queued behind them on sync.
            gt_sb = gtp.tile([P, NCHUNK * C], IO_DT)
            gt_v = gt.ap().rearrange("(jc p) r -> jc p r", p=P)
            for jc in range(NCHUNK):
                nc.scalar.dma_start(gt_sb[:, jc * C:(jc + 1) * C], gt_v[jc])

            act_v = act.ap().rearrange("n (jc p) m -> n jc p m", p=P)
            out_v = out.ap().rearrange("n (zc p) m -> n zc p m", p=P)

            for img in range(IMGS):
                a_sb = apool.tile([P, NCHUNK * HW], IO_DT)
                # column-block loads so matmuls start after the first block
                for cb in range(NCB):
                    for jc in range(NCHUNK):
                        nc.sync.dma_start(
                            a_sb[:, jc * HW + cb * CB: jc * HW + (cb + 1) * CB],
                            act_v[img, jc, :, cb * CB:(cb + 1) * CB])
                o_sbs = [opool.tile([P, HW], IO_DT, tag=f"o{zc}",
                                    name=f"o_sb{zc}")
                         for zc in range(NCHUNK)]
                # cb-outer: each 0.8MB column block is fully consumed (all
                # zc) before the next is needed, so the PE keeps pace with
                # the loads instead of stalling per-zc.
                for cb in range(NCB):
                    for zc in range(NCHUNK):
                        o_sb = o_sbs[zc]
                        for p2 in range(NPT // NCB):
                            p = cb * (NPT // NCB) + p2
                            ps = psp.tile([P, PT], mybir.dt.float32)
                            for i, d in enumerate(KEPT_D):
                                jc = (zc - d) % NCHUNK
                                nc.tensor.matmul(
                                    ps[:],
                                    gt_sb[:, jc * C + zc * P: jc * C + (zc + 1) * P],
                                    a_sb[:, jc * HW + p * PT: jc * HW + (p + 1) * PT],
                                    start=(i == 0), stop=(i == len(KEPT_D) - 1),
                                )
                            nc.vector.tensor_copy(
                                o_sb[:, p * PT:(p + 1) * PT], ps[:])
                        # store each finished column block immediately,
                        # alternating DMA rings to spread the drain
                        eng = nc.scalar if (cb + zc) % 2 else nc.sync
                        eng.dma_start(
                            out_v[img, zc, :, cb * CB:(cb + 1) * CB],
                            o_sb[:, cb * CB:(cb + 1) * CB])
    nc.compile()
    return nc


def _make_gt(inhib_kernel: np.ndarray) -> np.ndarray:
    k = np.asarray(inhib_kernel, dtype=np.float64)
    g = np.real(np.fft.ifft(1.0 / np.fft.fft(k)))
    gs = np.roll(g, -ROT)  # gs[t'] = g[(t'+ROT) mod C]
    idx = (np.arange(C)[None, :] - np.arange(C)[:, None]) % C
    return np.ascontiguousarray(gs[idx].astype(IO_NP))  # GTs[j, r]


def kernel(activations, inhib_kernel):
    acts = np.asarray(activations, dtype=np.float32)
    assert acts.shape == (N, C, H, W), acts.shape
    gt_np = _make_gt(np.asarray(inhib_kernel))

    if "nc" not in _CACHE:
        _CACHE["nc"] = _build_nc()
    nc = _CACHE["nc"]

    acts_h = acts.reshape(N, C, HW).astype(IO_NP)
    in_maps = [
        {"act": np.ascontiguousarray(acts_h[c * IMGS:(c + 1) * IMGS]),
         "gt": gt_np}
        for c in range(N_CORES)
    ]
    res = run_bass_kernel_spmd(nc, in_maps, core_ids=list(range(N_CORES)))
    z = np.concatenate([r["out"] for r in res.results], axis=0)
    # un-rotate: y[i] = z[(i - ROT) mod C], upcast to fp32
    y = z[:, (np.arange(C) - ROT) % C, :].astype(np.float32)
    return y.reshape(N, C, H, W)



# revision 3
# speedup vs baseline: 1.1381x; 1.1381x over previous
"""ConvergedInhibition TRN2 kernel.

The reference computes, per pixel (n,h,w), an FFT deconvolution along the
channel axis: y = ifft(fft(x)/fft(k)).real. Since k is fixed, this is a
circular convolution with g = ifft(1/fft(k)): y[i] = sum_j g[(i-j) mod C] x[j]
— a dense CxC circulant matmul applied to every pixel. Viewing activations[n]
as a [C, H*W] matrix A_n, the problem is out_n = G @ A_n: a [512,512] x
[512,3136] matmul per image, data-parallel over 32 images across 8 cores.

Implementation choices (measured on HW):
- The deconv kernel g is concentrated in a ~224-wide circular window.
  Rotating output rows by S=288 (z[r] = y[(r+S) mod C]) aligns the support
  so only 2 of 4 K-chunks of the contraction carry mass (each output row
  keeps a 256-wide sliding window of g; truncation costs ~2e-3 rel).
  The rotation is undone by a host-side gather.
- fp8 (e3m4) I/O: |x| < 6 << 15.5 = e3m4 max, 4 mantissa bits -> ~1.34e-2
  rms rounding per side. Per-jc input dtype and per-zc output dtype are
  configurable to trade error for HBM traffic (the ~358 GB/s per-core HBM
  limit is the roofline otherwise). Weights stay fp16 (PE upcasts operands
  to FP22 internally, mixed dtypes allowed).
- Only the 8 needed [128,128] weight blocks ship (256 KB, one DMA).
- PSUM->out casts are split between DVE (zc 0,1) and ACT (zc 2,3), each
  with its own completion semaphore, so the cast rate keeps up with the PE.
"""

import numpy as np
import ml_dtypes

import concourse.bass as bass  # noqa: F401  (registers bass types)
import concourse.mybir as mybir
from concourse import bacc
from concourse.bass_utils import run_bass_kernel_spmd

N_CORES = 8
N, C, H, W = 32, 512, 56, 56
HW = H * W                      # 3136
IMGS = N // N_CORES             # 4 images per core
P = 128                         # partitions
NCHUNK = C // P                 # 4
PT = 392                        # pixel tile (free dim), 3136 = 8*392
NPT = HW // PT                  # 8
CB = 784                        # DMA column block, 3136 = 4*784
NCB = HW // CB                  # 4
ROT = 288                       # output-row rotation aligning g's support
KEPT_D = (0, 1)                 # kept (zc - jc) mod 4 chunk distances
NKEPT = len(KEPT_D)

F8 = mybir.dt.float8e3
F16 = mybir.dt.float16
NP_OF = {F8: ml_dtypes.float8_e3m4, F16: np.float16}
IN_DT = [F8, F8, F8, F8]        # activation dtype per input chunk jc
OUT_DT = [F8, F8, F8, F8]       # output dtype per output chunk zc
W_DT = F16                      # gt weights
CAST_ENG = ["v", "v", "a", "a"]  # cast engine per zc (v=DVE, a=ACT)

P2 = NPT // NCB                       # p-tiles per column block (2)
TILES_PER_CB = NCHUNK * P2            # 8 psum tiles per (img, cb)
TILES_PER_IMG = NCB * TILES_PER_CB    # 32
NTILES = IMGS * TILES_PER_IMG         # 128


def tidx(img, cb, zc, p2):
    return img * TILES_PER_IMG + cb * TILES_PER_CB + zc * P2 + p2


def zc_of(t):
    return (t % TILES_PER_CB) // P2


# cnt_eng[e][t]: number of tiles t' <= t whose cast runs on engine e
_cnt = {"v": [0] * NTILES, "a": [0] * NTILES}
_c = {"v": 0, "a": 0}
for _t in range(NTILES):
    _c[CAST_ENG[zc_of(_t)]] += 1
    _cnt["v"][_t] = _c["v"]
    _cnt["a"][_t] = _c["a"]

_CACHE = {}


def _build_nc():
    """Hand-rolled engine programs with explicit semaphores.

    Streams:
      Sync:   gt + act loads (gt and (img0,cb0) prefetched in the clears
              block so their sems fire early) + half the stores
      Scalar: ACT casts for zc in {2,3} + the other half of the stores
      Tensor: 2-matmul PSUM groups per (img, cb, zc, p2) tile
      Vector: DVE casts for zc in {0,1}
    o_sb is per-(img, zc) (no reuse) so stores never gate casts; a_sb is
    double-buffered over images gated by s_mm; the 8 PSUM banks are a ring
    gated by the per-engine cast sems (bank t%8 is always cast by the same
    engine since t and t-8 share (zc, p2)).
    """
    nc = bacc.Bacc("TRN2", target_bir_lowering=False, debug=False,
                   num_devices=N_CORES)
    act = [nc.dram_tensor(f"act{jc}", [IMGS, P, HW], IN_DT[jc],
                          kind="ExternalInput") for jc in range(NCHUNK)]
    gtp = nc.dram_tensor("gtp", [P, NCHUNK * NKEPT * P], W_DT,
                         kind="ExternalInput")
    out = [nc.dram_tensor(f"out{zc}", [IMGS, P, HW], OUT_DT[zc],
                          kind="ExternalOutput") for zc in range(NCHUNK)]

    def store_ring(cb, zc):
        return "sync" if (cb + zc) % 2 == 0 else "scalar"

    from contextlib import ExitStack
    with ExitStack() as ctx:
        a_sb = [[ctx.enter_context(
            nc.sbuf_tensor(f"a_sb{h}_{jc}", [P, HW], IN_DT[jc])).ap()
            for jc in range(NCHUNK)] for h in range(2)]
        gt_sb = ctx.enter_context(
            nc.sbuf_tensor("gt_sb", [P, NCHUNK * NKEPT * P], W_DT)).ap()
        o_sb = [[ctx.enter_context(
            nc.sbuf_tensor(f"o_sb{i}_{z}", [P, HW], OUT_DT[z])).ap()
            for z in range(NCHUNK)] for i in range(IMGS)]
        psum = [ctx.enter_context(
            nc.psum_tensor(f"ps{i}", [P, 512], mybir.dt.float32)).ap()
            for i in range(8)]

        s_gt = nc.alloc_semaphore("s_gt")
        s_ld = [[nc.alloc_semaphore(f"s_ld{h}_{cb}") for cb in range(NCB)]
                for h in range(2)]
        s_mm = nc.alloc_semaphore("s_mm")
        s_cast = {"v": nc.alloc_semaphore("s_cast_v"),
                  "a": nc.alloc_semaphore("s_cast_a")}
        s_st = {"sync": nc.alloc_semaphore("s_st_sync"),
                "scalar": nc.alloc_semaphore("s_st_sca")}
        all_sems = ([s_gt, s_mm, s_cast["v"], s_cast["a"],
                     s_st["sync"], s_st["scalar"]]
                    + [s for row in s_ld for s in row])

        def emit_loads(sync, img, cb):
            if img >= 2:
                sync.wait_ge(s_mm, TILES_PER_IMG * (img - 2)
                             + TILES_PER_CB * (cb + 1))
            for jc in range(NCHUNK):
                sync.dma_start(
                    a_sb[img % 2][jc][:, cb * CB:(cb + 1) * CB],
                    act[jc].ap()[img, :, cb * CB:(cb + 1) * CB],
                ).then_inc(s_ld[img % 2][cb], 16)

        def emit_store(eng, ring, img, cb, zc):
            e = CAST_ENG[zc]
            eng.wait_ge(s_cast[e], _cnt[e][tidx(img, cb, zc, P2 - 1)])
            eng.dma_start(
                out[zc].ap()[img, :, cb * CB:(cb + 1) * CB],
                o_sb[img][zc][:, cb * CB:(cb + 1) * CB],
            ).then_inc(s_st[ring], 16)

        # Stage 0: clear semaphores, then prefetch gt + (img0, cb0) on the
        # sync queue (ordered after the clears on that queue; their sem
        # increments land while the main block is still in its barrier).
        # Sems are NOT zeroed on alloc and must not carry values across
        # executions, hence the clears.
        with nc.Block("clears") as blk:

            @blk.sync
            def _(sync):
                for s in all_sems:
                    sync.sem_clear(s)
                sync.dma_start(gt_sb[:], gtp.ap()[:]).then_inc(s_gt, 16)
                emit_loads(sync, 0, 0)

        with nc.Block("main") as blk:

            @blk.sync
            def _(sync):
                n_store = 0
                for cb in range(1, NCB):
                    emit_loads(sync, 0, cb)
                for cb in range(NCB):
                    emit_loads(sync, 1, cb)
                for img in range(IMGS):
                    for cb in range(NCB):
                        for zc in range(NCHUNK):
                            if store_ring(cb, zc) != "sync":
                                continue
                            emit_store(sync, "sync", img, cb, zc)
                            n_store += 1
                        if img + 2 < IMGS:
                            emit_loads(sync, img + 2, cb)
                sync.wait_ge(s_st["sync"], 16 * n_store)

            @blk.scalar
            def _(scalar):
                # ACT casts for its zc set, with its stores woven in right
                # after the tile they depend on (the t order makes every
                # wait monotone).
                pend = []
                for img in range(IMGS):
                    for cb in range(NCB):
                        for zc in range(NCHUNK):
                            if store_ring(cb, zc) == "scalar":
                                pend.append((tidx(img, cb, zc, P2 - 1),
                                             img, cb, zc))
                pend.sort()
                pi = 0
                n_store = 0
                for t in range(NTILES):
                    zc = zc_of(t)
                    if CAST_ENG[zc] == "a":
                        scalar.wait_ge(s_mm, t + 1)
                        img, cb = t // TILES_PER_IMG, \
                            (t % TILES_PER_IMG) // TILES_PER_CB
                        p = cb * P2 + (t % P2)
                        scalar.copy(
                            o_sb[img][zc][:, p * PT:(p + 1) * PT],
                            psum[t % 8][:, :PT],
                        ).then_inc(s_cast["a"])
                    while pi < len(pend) and pend[pi][0] <= t:
                        _, img, cb, zc = pend[pi]
                        emit_store(scalar, "scalar", img, cb, zc)
                        n_store += 1
                        pi += 1
                for j in range(pi, len(pend)):
                    _, img, cb, zc = pend[j]
                    emit_store(scalar, "scalar", img, cb, zc)
                    n_store += 1
                scalar.wait_ge(s_st["scalar"], 16 * n_store)

            @blk.tensor
            def _(tensor):
                tensor.wait_ge(s_gt, 16)
                for img in range(IMGS):
                    for cb in range(NCB):
                        tensor.wait_ge(s_ld[img % 2][cb],
                                       64 * (img // 2 + 1))
                        for zc in range(NCHUNK):
                            for p2 in range(P2):
                                t = tidx(img, cb, zc, p2)
                                if t >= 8:
                                    e = CAST_ENG[zc]
                                    tensor.wait_ge(s_cast[e],
                                                   _cnt[e][t - 8])
                                p = cb * P2 + p2
                                for i, d in enumerate(KEPT_D):
                                    jc = (zc - d) % NCHUNK
                                    mm = tensor.matmul(
                                        psum[t % 8][:, :PT],
                                        gt_sb[:, (zc * NKEPT + i) * P:
                                              (zc * NKEPT + i + 1) * P],
                                        a_sb[img % 2][jc][
                                            :, p * PT:(p + 1) * PT],
                                        start=(i == 0), stop=(i == NKEPT - 1),
                                    )
                                mm.then_inc(s_mm)

            @blk.vector
            def _(vector):
                for t in range(NTILES):
                    zc = zc_of(t)
                    if CAST_ENG[zc] != "v":
                        continue
                    vector.wait_ge(s_mm, t + 1)
                    img, cb = t // TILES_PER_IMG, \
                        (t % TILES_PER_IMG) // TILES_PER_CB
                    p = cb * P2 + (t % P2)
                    vector.tensor_copy(
                        o_sb[img][zc][:, p * PT:(p + 1) * PT],
                        psum[t % 8][:, :PT],
                    ).then_inc(s_cast["v"])

    nc.compile()
    return nc


def _make_gt(inhib_kernel: np.ndarray) -> np.ndarray:
    """Packed stationary blocks: col block (zc*NKEPT+i) holds
    GTs[jc*P:(jc+1)*P, zc*P:(zc+1)*P] with jc=(zc-KEPT_D[i])%NCHUNK,
    where GTs[j, r] = g[(r + ROT - j) mod C]."""
    k = np.asarray(inhib_kernel, dtype=np.float64)
    g = np.real(np.fft.ifft(1.0 / np.fft.fft(k)))
    gts = g[(np.arange(C)[None, :] + ROT - np.arange(C)[:, None]) % C]
    gtp = np.empty((P, NCHUNK * NKEPT * P), dtype=NP_OF[W_DT])
    for zc in range(NCHUNK):
        for i, d in enumerate(KEPT_D):
            jc = (zc - d) % NCHUNK
            b = zc * NKEPT + i
            gtp[:, b * P:(b + 1) * P] = gts[jc * P:(jc + 1) * P,
                                            zc * P:(zc + 1) * P]
    return np.ascontiguousarray(gtp)


def make_in_maps(activations, inhib_kernel):
    acts = np.asarray(activations, dtype=np.float32).reshape(N, C, HW)
    gtp = _make_gt(np.asarray(inhib_kernel))
    in_maps = []
    for c in range(N_CORES):
        m = {"gtp": gtp}
        sl = acts[c * IMGS:(c + 1) * IMGS]
        for jc in range(NCHUNK):
            m[f"act{jc}"] = np.ascontiguousarray(
                sl[:, jc * P:(jc + 1) * P]).astype(NP_OF[IN_DT[jc]])
        in_maps.append(m)
    return in_maps


def kernel(activations, inhib_kernel):
    acts = np.asarray(activations, dtype=np.float32)
    assert acts.shape == (N, C, H, W), acts.shape

    if "nc" not in _CACHE:
        _CACHE["nc"] = _build_nc()
    nc = _CACHE["nc"]

    in_maps = make_in_maps(acts, inhib_kernel)
    res = run_bass_kernel_spmd(nc, in_maps, core_ids=list(range(N_CORES)))
    z = np.concatenate(
        [np.concatenate([r[f"out{zc}"].astype(np.float32)
                         for zc in range(NCHUNK)], axis=1)
         for r in res.results], axis=0)
    # un-rotate: y[i] = z[(i - ROT) mod C]
    y = z[:, (np.arange(C) - ROT) % C, :]
    return y.reshape(N, C, H, W)


# revision 4
# speedup vs baseline: 1.4055x; 1.2349x over previous
"""ConvergedInhibition TRN2 kernel.

The reference computes, per pixel (n,h,w), an FFT deconvolution along the
channel axis: y = ifft(fft(x)/fft(k)).real. Since k is fixed, this is a
circular convolution with g = ifft(1/fft(k)): y[i] = sum_j g[(i-j) mod C] x[j]
— a dense CxC circulant matmul applied to every pixel. Viewing activations[n]
as a [C, H*W] matrix A_n, the problem is out_n = G @ A_n: a [512,512] x
[512,3136] matmul per image, data-parallel over 32 images across 8 cores.

Implementation choices (measured on HW):
- The deconv kernel g is concentrated in a ~224-wide circular window.
  Rotating output rows by S=288 (z[r] = y[(r+S) mod C]) aligns the support
  so only 2 of 4 K-chunks of the contraction carry mass (each output row
  keeps a 256-wide sliding window of g; truncation costs ~2e-3 rel).
  The rotation is undone by a host-side gather.
- fp8 (e3m4) I/O: |x| < 6 << 15.5 = e3m4 max, 4 mantissa bits -> ~1.34e-2
  rms rounding per side (measured end-to-end rel err 1.907e-2, HW matches
  the numpy simulation exactly). Per-zc output dtype stays configurable.
  Weights are fp16 (PE upcasts operands to FP22, mixed dtypes allowed).
- Only the 8 needed [128,128] weight blocks ship (256 KB, one DMA).
- Each dma_start occupies its HWDGE ring ~630ns regardless of size, so
  DMAs are as large as possible: one per (img, cb) on loads (400KB, 3D
  access pattern covering all 4 channel chunks), half-image-width stores.
  All loads+stores ride the sync ring; gt + the first block are prefetched
  from the semaphore-clears block so their sems fire before the main
  block's barrier lifts.
- PSUM->out casts are split between DVE (zc 0,1) and ACT (zc 2,3), each
  with its own completion semaphore (bank t%8 is always cast by the same
  engine since t and t-8 share (zc, p2)), so cast rate keeps up with PE.
"""

import numpy as np
import ml_dtypes

import concourse.bass as bass  # noqa: F401  (registers bass types)
import concourse.mybir as mybir
from concourse import bacc
from concourse.bass_utils import run_bass_kernel_spmd

N_CORES = 8
N, C, H, W = 32, 512, 56, 56
HW = H * W                      # 3136
IMGS = N // N_CORES             # 4 images per core
P = 128                         # partitions
NCHUNK = C // P                 # 4
PT = 392                        # pixel tile (free dim), 3136 = 8*392
NPT = HW // PT                  # 8
CB = 784                        # column block, 3136 = 4*784
NCB = HW // CB                  # 4
ROT = 288                       # output-row rotation aligning g's support
KEPT_D = (0, 1)                 # kept (zc - jc) mod 4 chunk distances
NKEPT = len(KEPT_D)

F8 = mybir.dt.float8e3
F16 = mybir.dt.float16
NP_OF = {F8: ml_dtypes.float8_e3m4, F16: np.float16}
A_DT = F8                       # activation (input) dtype
OUT_DT = [F8, F8, F8, F8]       # output dtype per output chunk zc
W_DT = F16                      # gt weights
CAST_ENG = ["v", "v", "a", "a"]  # cast engine per zc (v=DVE, a=ACT)

P2 = NPT // NCB                       # p-tiles per column block (2)
TILES_PER_CB = NCHUNK * P2            # 8 psum tiles per (img, cb)
TILES_PER_IMG = NCB * TILES_PER_CB    # 32
NTILES = IMGS * TILES_PER_IMG         # 128


def tidx(img, cb, zc, p2):
    return img * TILES_PER_IMG + cb * TILES_PER_CB + zc * P2 + p2


def zc_of(t):
    return (t % TILES_PER_CB) // P2


# cnt_eng[e][t]: number of tiles t' <= t whose cast runs on engine e
_cnt = {"v": [0] * NTILES, "a": [0] * NTILES}
_c = {"v": 0, "a": 0}
for _t in range(NTILES):
    _c[CAST_ENG[zc_of(_t)]] += 1
    _cnt["v"][_t] = _c["v"]
    _cnt["a"][_t] = _c["a"]

_CACHE = {}


def _build_nc():
    nc = bacc.Bacc("TRN2", target_bir_lowering=False, debug=False,
                   num_devices=N_CORES)
    act = nc.dram_tensor("act", [IMGS, C, HW], A_DT, kind="ExternalInput")
    gtp = nc.dram_tensor("gtp", [P, NCHUNK * NKEPT * P], W_DT,
                         kind="ExternalInput")
    out = [nc.dram_tensor(f"out{zc}", [IMGS, P, HW], OUT_DT[zc],
                          kind="ExternalOutput") for zc in range(NCHUNK)]

    # [img, p, jc, m]: partition-major view of the (jc p) channel split so
    # one DMA per (img, cb) moves all 4 chunks
    act_v = act.ap().rearrange("n (jc p) m -> n p jc m", p=P)

    from contextlib import ExitStack
    with ExitStack() as ctx:
        a_sb = [ctx.enter_context(
            nc.sbuf_tensor(f"a_sb{h}", [P, NCHUNK * HW], A_DT)).ap()
            for h in range(2)]
        a_sb_v = [a.rearrange("p (jc m) -> p jc m", m=HW) for a in a_sb]
        gt_sb = ctx.enter_context(
            nc.sbuf_tensor("gt_sb", [P, NCHUNK * NKEPT * P], W_DT)).ap()
        o_sb = [[ctx.enter_context(
            nc.sbuf_tensor(f"o_sb{i}_{z}", [P, HW], OUT_DT[z])).ap()
            for z in range(NCHUNK)] for i in range(IMGS)]
        psum = [ctx.enter_context(
            nc.psum_tensor(f"ps{i}", [P, 512], mybir.dt.float32)).ap()
            for i in range(8)]

        s_gt = nc.alloc_semaphore("s_gt")
        s_ld = [[nc.alloc_semaphore(f"s_ld{h}_{cb}") for cb in range(NCB)]
                for h in range(2)]
        s_mm = nc.alloc_semaphore("s_mm")
        s_cast = {"v": nc.alloc_semaphore("s_cast_v"),
                  "a": nc.alloc_semaphore("s_cast_a")}
        s_st = nc.alloc_semaphore("s_st")
        all_sems = ([s_gt, s_mm, s_cast["v"], s_cast["a"], s_st]
                    + [s for row in s_ld for s in row])

        def emit_load(sync, img, cb):
            # one DMA: [128 part, 4 jc, 784 cols]
            if img >= 2:
                sync.wait_ge(s_mm, TILES_PER_IMG * (img - 2)
                             + TILES_PER_CB * (cb + 1))
            sync.dma_start(
                a_sb_v[img % 2][:, :, cb * CB:(cb + 1) * CB],
                act_v[img, :, :, cb * CB:(cb + 1) * CB],
            ).then_inc(s_ld[img % 2][cb], 16)

        def emit_store(sync, img, zc, h2):
            # half-image-width store: cbs {2*h2, 2*h2+1}
            e = CAST_ENG[zc]
            sync.wait_ge(s_cast[e],
                         _cnt[e][tidx(img, 2 * h2 + 1, zc, P2 - 1)])
            sync.dma_start(
                out[zc].ap()[img, :, h2 * 2 * CB:(h2 + 1) * 2 * CB],
                o_sb[img][zc][:, h2 * 2 * CB:(h2 + 1) * 2 * CB],
            ).then_inc(s_st, 16)

        # Stage 0: clear semaphores, then prefetch gt + (img0, cb0) on the
        # sync queue (ordered after the clears on that queue; their sem
        # increments land while the main block is still in its barrier).
        # Sems are NOT zeroed on alloc and must not carry values across
        # executions, hence the clears.
        with nc.Block("clears") as blk:

            @blk.sync
            def _(sync):
                for s in all_sems:
                    sync.sem_clear(s)
                sync.dma_start(gt_sb[:], gtp.ap()[:]).then_inc(s_gt, 16)
                emit_load(sync, 0, 0)

        with nc.Block("main") as blk:

            @blk.sync
            def _(sync):
                for cb in range(1, NCB):
                    emit_load(sync, 0, cb)
                for cb in range(NCB):
                    emit_load(sync, 1, cb)
                n_store = 0
                for img in range(IMGS):
                    for h2 in range(NCB // 2):
                        if img + 2 < IMGS:
                            emit_load(sync, img + 2, 2 * h2)
                            emit_load(sync, img + 2, 2 * h2 + 1)
                        for zc in range(NCHUNK):
                            emit_store(sync, img, zc, h2)
                            n_store += 1
                sync.wait_ge(s_st, 16 * n_store)

            @blk.scalar
            def _(scalar):
                for t in range(NTILES):
                    zc = zc_of(t)
                    if CAST_ENG[zc] != "a":
                        continue
                    scalar.wait_ge(s_mm, t + 1)
                    img, cb = t // TILES_PER_IMG, \
                        (t % TILES_PER_IMG) // TILES_PER_CB
                    p = cb * P2 + (t % P2)
                    scalar.copy(
                        o_sb[img][zc][:, p * PT:(p + 1) * PT],
                        psum[t % 8][:, :PT],
                    ).then_inc(s_cast["a"])

            @blk.tensor
            def _(tensor):
                tensor.wait_ge(s_gt, 16)
                for img in range(IMGS):
                    for cb in range(NCB):
                        tensor.wait_ge(s_ld[img % 2][cb],
                                       16 * (img // 2 + 1))
                        for zc in range(NCHUNK):
                            for p2 in range(P2):
                                t = tidx(img, cb, zc, p2)
                                if t >= 8:
                                    e = CAST_ENG[zc]
                                    tensor.wait_ge(s_cast[e],
                                                   _cnt[e][t - 8])
                                p = cb * P2 + p2
                                for i, d in enumerate(KEPT_D):
                                    jc = (zc - d) % NCHUNK
                                    mm = tensor.matmul(
                                        psum[t % 8][:, :PT],
                                        gt_sb[:, (zc * NKEPT + i) * P:
                                              (zc * NKEPT + i + 1) * P],
                                        a_sb_v[img % 2][
                                            :, jc, p * PT:(p + 1) * PT],
                                        start=(i == 0), stop=(i == NKEPT - 1),
                                    )
                                mm.then_inc(s_mm)

            @blk.vector
            def _(vector):
                for t in range(NTILES):
                    zc = zc_of(t)
                    if CAST_ENG[zc] != "v":
                        continue
                    vector.wait_ge(s_mm, t + 1)
                    img, cb = t // TILES_PER_IMG, \
                        (t % TILES_PER_IMG) // TILES_PER_CB
                    p = cb * P2 + (t % P2)
                    vector.tensor_copy(
                        o_sb[img][zc][:, p * PT:(p + 1) * PT],
                        psum[t % 8][:, :PT],
                    ).then_inc(s_cast["v"])

    nc.compile()
    return nc


def _make_gt(inhib_kernel: np.ndarray) -> np.ndarray:
    """Packed stationary blocks: col block (zc*NKEPT+i) holds
    GTs[jc*P:(jc+1)*P, zc*P:(zc+1)*P] with jc=(zc-KEPT_D[i])%NCHUNK,
    where GTs[j, r] = g[(r + ROT - j) mod C]."""
    k = np.asarray(inhib_kernel, dtype=np.float64)
    g = np.real(np.fft.ifft(1.0 / np.fft.fft(k)))
    gts = g[(np.arange(C)[None, :] + ROT - np.arange(C)[:, None]) % C]
    gtp = np.empty((P, NCHUNK * NKEPT * P), dtype=NP_OF[W_DT])
    for zc in range(NCHUNK):
        for i, d in enumerate(KEPT_D):
            jc = (zc - d) % NCHUNK
            b = zc * NKEPT + i
            gtp[:, b * P:(b + 1) * P] = gts[jc * P:(jc + 1) * P,
                                            zc * P:(zc + 1) * P]
    return np.ascontiguousarray(gtp)


def make_in_maps(activations, inhib_kernel):
    acts = np.asarray(activations, dtype=np.float32).reshape(N, C, HW)
    acts8 = acts.astype(NP_OF[A_DT])
    gtp = _make_gt(np.asarray(inhib_kernel))
    return [
        {"act": np.ascontiguousarray(acts8[c * IMGS:(c + 1) * IMGS]),
         "gtp": gtp}
        for c in range(N_CORES)
    ]


def kernel(activations, inhib_kernel):
    acts = np.asarray(activations, dtype=np.float32)
    assert acts.shape == (N, C, H, W), acts.shape

    if "nc" not in _CACHE:
        _CACHE["nc"] = _build_nc()
    nc = _CACHE["nc"]

    in_maps = make_in_maps(acts, inhib_kernel)
    res = run_bass_kernel_spmd(nc, in_maps, core_ids=list(range(N_CORES)))
    z = np.concatenate(
        [np.concatenate([r[f"out{zc}"].astype(np.float32)
                         for zc in range(NCHUNK)], axis=1)
         for r in res.results], axis=0)
    # un-rotate: y[i] = z[(i - ROT) mod C]
    y = z[:, (np.arange(C) - ROT) % C, :]
    return y.reshape(N, C, H, W)
